# revision 20
# baseline (speedup 1.0000x reference)
"""GATv2-style 2-layer GNN (DirVGAEEncoder) on 8 Trainium2 NeuronCores.

Strategy (edge-parallel, dst-sharded):
- Nodes are assigned to cores round-robin by in-degree rank; within a core,
  nodes are sorted by (degA, degB) and cut into blocks of 128 (degree
  bucketing keeps slot padding low). A node owns one SBUF partition in its
  block; its in-edges occupy "slot" columns along the free dim.
- Node GEMMs run data-parallel; the att-scaled source table is AllGather'd
  so every core can gather arbitrary source rows with dma_gather (int16
  indices => the table is addressed through two <=32K-row views; each
  node's edges split into A-slots (src on cores 0-3) and B-slots (4-7)).
  Gathers round-robin the 4 SWDGE queues so all 8 GpSimd DSP cores
  generate descriptors concurrently.
- Within a superbatch all blocks share one region length per side
  (LA/LB = max over blocks), so every elementwise/reduce op covers the
  whole region in ONE instruction: u += xr (broadcast), v = Lrelu trick,
  logit = reduce_c(v), w = exp(logit), den = reduce_l(w), vals = u*w
  (in place), num = reduce_l(vals) (strided).
- Leaky-relu fold: logit = sum_pos 5*lrelu_{0.2}(u) + sum_neg lrelu_{5}(u)
  with u = 0.2*att*(xl+xr) (att folded into the GEMM weights, channels
  sorted by sign of att). One channel reduce instead of three.
- Padded slots gather a dedicated -1000 pad row, so their exp underflows
  to zero: no mask tensor at all.
- Layer-1 numerator: sum_l w*u = att-scaled num  =>  true numerator =
  (num - den*xr) / (0.2*att). The layer-2 table carries [att2*hl | hl] so
  its numerator uses plain hl directly.
- No softmax max-subtraction: logits are O(1) here and the max cancels
  exactly in the reference formula, so exp() is safe.
"""
import sys

sys.path.insert(0, "/opt/trn_rl_repo")
import numpy as np

P = 128
CORES = 8
NEG = 0.2
GW = 64  # gather row width (fp32) for both layers -> 256B rows
POISON1 = -0.25  # pad-row value, layer 1: pad logits ~ -50
POISON2 = -0.5   # pad-row value, layer 2: pad logits ~ -54


# ---------------------------------------------------------------- host prep
def _plan(edge_index, N, slmax):
    src = edge_index[0].astype(np.int64)
    dst = edge_index[1].astype(np.int64)
    E = src.shape[0]
    NPC = N // CORES
    NBLK = (NPC + P - 1) // P
    NPC_PAD = NBLK * P
    TROWS = CORES * NPC_PAD
    SPLIT = 4 * NPC_PAD
    assert SPLIT <= 32768
    assert NPC < NPC_PAD  # pad row (local index NPC) must exist

    deg = np.bincount(dst, minlength=N)
    order = np.argsort(-deg, kind="stable")  # rank -> node
    core_of = np.empty(N, np.int64)
    core_of[order] = np.arange(N) % CORES
    isA = core_of[src] < 4
    degA = np.bincount(dst[isA], minlength=N)
    degB = deg - degA

    # within-core (degA, degB) sort -> local index; pos = table row
    pos = np.empty(N, np.int64)
    for k in range(CORES):
        nodes = order[core_of[order] == k]
        nodes = nodes[np.lexsort(
            (-degB[nodes], -((degA[nodes] + 1) // 2 * 2))
        )]
        pos[nodes] = k * NPC_PAD + np.arange(len(nodes))

    local = pos % NPC_PAD
    blk_of = local // P
    part_of = local % P

    # per-block max degrees (block schedule shared across cores)
    bLA = np.ones(NBLK, np.int64)
    bLB = np.ones(NBLK, np.int64)
    np.maximum.at(bLA, blk_of, degA)
    np.maximum.at(bLB, blk_of, degB)

    # superbatches: consecutive blocks, equalized LA/LB inside each sb,
    # with nb*(LA+LB) <= slmax. DP-optimal split (small per-sb penalty
    # trades a little padding for far fewer superbatches).
    SB_PEN = 8
    NB_CAP = 6
    INF = 1 << 40
    best = [INF] * (NBLK + 1)
    best[0] = 0
    prev = [0] * (NBLK + 1)
    for j in range(1, NBLK + 1):
        la = lb = 0
        for i in range(j, 0, -1):
            if j - i + 1 > NB_CAP:
                break
            la = max(la, int(bLA[i - 1]))
            lb = max(lb, int(bLB[i - 1]))
            cost = (j - i + 1) * (la + la % 2 + lb + lb % 2)
            if cost > slmax:
                break
            if best[i - 1] + cost + SB_PEN < best[j]:
                best[j] = best[i - 1] + cost + SB_PEN
                prev[j] = i - 1
    sbs = []
    j = NBLK
    while j > 0:
        i = prev[j]
        sbs.append(list(range(i, j)))
        j = i
    sbs.reverse()

    # column layout per sb: [A region nb*LA | B region nb*LB], block-major
    colA = np.zeros(NBLK, np.int64)
    colB = np.zeros(NBLK, np.int64)
    sb_meta = []  # (c0, LA, LB, blocks)
    c = 0
    for blocks in sbs:
        la = max(bLA[b] for b in blocks)
        la += la % 2
        lb = max(bLB[b] for b in blocks)
        lb += lb % 2
        nb = len(blocks)
        for bi, b in enumerate(blocks):
            colA[b] = c + bi * la
            colB[b] = c + nb * la + bi * lb
        sb_meta.append((c, int(la), int(lb), list(blocks)))
        c += nb * (la + lb)
    SL = c

    # per-core edge -> (partition, column) via grouped slot ranking
    idx2d = np.full((CORES, P, SL), NPC, np.int64)  # default: pad row
    ek = core_of[dst]
    key = pos[dst] * 2 + (~isA).astype(np.int64)
    eorder = np.argsort(key, kind="stable")
    ksorted = key[eorder]
    grp_start = np.r_[0, np.flatnonzero(np.diff(ksorted)) + 1]
    slot_sorted = np.arange(E) - np.repeat(
        grp_start, np.diff(np.r_[grp_start, E])
    )
    slot = np.empty(E, np.int64)
    slot[eorder] = slot_sorted

    col = np.where(isA, colA[blk_of[dst]], colB[blk_of[dst]]) + slot
    rowval = np.where(isA, pos[src], pos[src] - SPLIT)
    idx2d[ek, part_of[dst], col] = rowval

    def wrap_region(core, c0, width):
        arr = idx2d[core][:, c0 : c0 + width]  # [P, W]
        flat = arr.T.ravel()  # i = c*128 + p
        w = flat.reshape(-1, 16).T.astype(np.int16)  # [16, W*8]
        return np.tile(w, (8, 1))  # [128, W*8]

    ca = cb = 0
    for (c0, la, lb, blocks) in sb_meta:
        nb = len(blocks)
        ca += nb * la * 8
        cb += nb * lb * 8
    idxA = np.zeros((CORES, P, ca), np.int16)
    idxB = np.zeros((CORES, P, cb), np.int16)
    for k in range(CORES):
        pa, pb = [], []
        for (c0, la, lb, blocks) in sb_meta:
            nb = len(blocks)
            pa.append(wrap_region(k, c0, nb * la))
            pb.append(wrap_region(k, c0 + nb * la, nb * lb))
        idxA[k] = np.concatenate(pa, axis=1)
        idxB[k] = np.concatenate(pb, axis=1)

    return dict(
        N=N, E=E, NPC=NPC, NBLK=NBLK, NPC_PAD=NPC_PAD, TROWS=TROWS,
        SPLIT=SPLIT, SL=SL, colA=colA, colB=colB,
        sb_meta=sb_meta, pos=pos, idxA=idxA, idxB=idxB, slmax=slmax,
    )


def _prep_weights(W1_l, W1_r, att1, b1, W2_l, W2_r, att2, b2):
    s1 = np.argsort(att1 <= 0, kind="stable")  # att1>0 channels first
    npos1 = int((att1 > 0).sum())
    s2 = np.argsort(att2 <= 0, kind="stable")
    npos2 = int((att2 > 0).sum())

    W1l_s = (0.2 * W1_l * att1[None, :])[:, s1].astype(np.float32)
    W1r_s = (0.2 * W1_r * att1[None, :])[:, s1].astype(np.float32)
    W1cat = np.ascontiguousarray(np.concatenate([W1l_s, W1r_s], axis=1))

    inv1 = (5.0 / att1[s1]).astype(np.float32)
    b1_s = b1[s1].astype(np.float32)

    W2l_p = W2_l[s1, :]
    W2r_p = W2_r[s1, :]
    W2cat = np.ascontiguousarray(np.concatenate(
        [(0.2 * W2l_p * att2[None, :])[:, s2], W2l_p,
         (0.2 * W2r_p * att2[None, :])[:, s2]], axis=1,
    ).astype(np.float32))
    return dict(W1cat=W1cat, W2cat=W2cat, inv1=inv1, b1=b1_s,
                b2=b2.astype(np.float32), npos1=npos1, npos2=npos2)


# ------------------------------------------------------------- bass builder
def _build(plan, wp, IN_C, H, O, debug):
    from concourse import bass, mybir, tile, bacc
    from concourse.masks import make_identity

    f32 = mybir.dt.float32
    i16 = mybir.dt.int16
    AF = mybir.ActivationFunctionType
    AX = mybir.AxisListType
    NBLK, NPC_PAD, TROWS, SPLIT, SL = (
        plan["NBLK"], plan["NPC_PAD"], plan["TROWS"], plan["SPLIT"],
        plan["SL"],
    )
    NPC = plan["NPC"]
    sb_meta = plan["sb_meta"]
    npos1, npos2 = wp["npos1"], wp["npos2"]
    MA = plan["idxA"].shape[2]
    MB = plan["idxB"].shape[2]
    SLMAX = plan["slmax"]
    NBMAX = max(len(m[3]) for m in sb_meta)
    assert H <= GW and 2 * O <= GW
    PAD_BLK = NPC // P          # block holding the pad rows
    PAD_P0 = NPC % P            # first pad partition in that block
    NPAD = NPC_PAD - NPC        # number of pad rows

    nc = bacc.Bacc("TRN2", target_bir_lowering=False, debug=debug,
                   num_devices=CORES, num_swdge_queues=4)

    xT = nc.dram_tensor("xT", [IN_C, NPC_PAD], f32, kind="ExternalInput")
    W1cat_d = nc.dram_tensor("W1cat", [IN_C, 2 * H], f32, kind="ExternalInput")
    W2cat_d = nc.dram_tensor("W2cat", [H, 3 * O], f32, kind="ExternalInput")
    inv1_d = nc.dram_tensor("inv1", [P, H], f32, kind="ExternalInput")
    b1_d = nc.dram_tensor("b1", [P, H], f32, kind="ExternalInput")
    b2_d = nc.dram_tensor("b2", [P, O], f32, kind="ExternalInput")
    idxA_d = nc.dram_tensor("idxA", [P, MA], i16, kind="ExternalInput")
    idxB_d = nc.dram_tensor("idxB", [P, MB], i16, kind="ExternalInput")
    alpha_d = nc.dram_tensor("alpha", [NPC_PAD, O], f32, kind="ExternalOutput")

    t1_in = nc.dram_tensor("t1_in", [NPC_PAD, H], f32)
    t2_in = nc.dram_tensor("t2_in", [NPC_PAD, 2 * O], f32)
    table1 = nc.dram_tensor("table1", [TROWS, H], f32, addr_space="Shared")
    table2 = nc.dram_tensor("table2", [TROWS, 2 * O], f32, addr_space="Shared")

    groups = [list(range(CORES))]

    with tile.TileContext(nc) as tc:
        with (
            tc.tile_pool(name="gath", bufs=3) as gath,
            tc.tile_pool(name="scratch", bufs=1) as scratch,
            tc.tile_pool(name="keep", bufs=1) as keep,
            tc.tile_pool(name="small", bufs=2) as small,
            tc.tile_pool(name="psum", bufs=4, space="PSUM") as psum,
        ):
            # persistent tiles
            w1 = keep.tile([IN_C, 2 * H], f32)
            nc.sync.dma_start(out=w1[:], in_=W1cat_d[:])
            w2 = keep.tile([H, 3 * O], f32)
            nc.sync.dma_start(out=w2[:], in_=W2cat_d[:])
            inv1_t = keep.tile([P, H], f32)
            nc.sync.dma_start(out=inv1_t[:], in_=inv1_d[:])
            b1_t = keep.tile([P, H], f32)
            nc.sync.dma_start(out=b1_t[:], in_=b1_d[:])
            b2_t = keep.tile([P, O], f32)
            nc.sync.dma_start(out=b2_t[:], in_=b2_d[:])
            iA = keep.tile([P, MA], i16)
            nc.sync.dma_start(out=iA[:], in_=idxA_d[:])
            iB = keep.tile([P, MB], i16)
            nc.sync.dma_start(out=iB[:], in_=idxB_d[:])
            xT_t = scratch.tile([IN_C, NPC_PAD], f32, tag="t")
            nc.sync.dma_start(out=xT_t[:], in_=xT[:])
            ident = keep.tile([P, P], f32)
            make_identity(nc, ident[:])

            gem1 = keep.tile([P, NBLK, 2 * H], f32)  # [att1*xl | att1*xr]
            gem2 = keep.tile([P, NBLK, 3 * O], f32)
            hT = keep.tile([H, NBLK * P], f32)
            poison = keep.tile([P, 2 * GW], f32)  # pad-row sources
            nc.scalar.activation(
                out=poison[:, 0:GW], in_=b1_t[:, 0:GW],
                func=AF.Copy, scale=0.0, bias=POISON1,
            )
            nc.scalar.activation(
                out=poison[:, GW : 2 * GW], in_=b1_t[:, 0:GW],
                func=AF.Copy, scale=0.0, bias=POISON2,
            )

            # ---------------- GEMM 1 + AllGather table1
            for b in range(NBLK):
                ps = psum.tile([P, 2 * H], f32, tag="gemm_ps")
                nc.tensor.matmul(
                    out=ps[:], lhsT=xT_t[:, b * P : (b + 1) * P], rhs=w1[:],
                    start=True, stop=True,
                )
                nc.vector.tensor_copy(out=gem1[:, b, :], in_=ps[:])
            # write table input: full blocks, the partial last block, and
            # BIG_NEG poison for the pad rows (disjoint row ranges)
            nc.sync.dma_start(
                out=t1_in.ap()[0 : PAD_BLK * P, :]
                .rearrange("(b p) h -> p b h", p=P),
                in_=gem1[:, 0:PAD_BLK, 0:H],
            )
            nc.sync.dma_start(
                out=t1_in.ap()[PAD_BLK * P : NPC, :],
                in_=gem1[0:PAD_P0, PAD_BLK, 0:H],
            )
            nc.sync.dma_start(
                out=t1_in.ap()[NPC:NPC_PAD, :],
                in_=poison[0:NPAD, 0:H],
            )
            nc.gpsimd.collective_compute(
                "AllGather", mybir.AluOpType.bypass, replica_groups=groups,
                ins=[t1_in[:].opt()], outs=[table1[:].opt()],
            )

            # ---------------- generic edge phase
            def edge_phase(layer, table, gem, C, nposL, out_hook):
                v0 = 0 if layer == 1 else O  # numerator columns v0:v0+C
                xr0 = H if layer == 1 else 2 * O  # xr column in gem
                ca0 = cb0 = 0
                for si, (c0, la, lb, blocks) in enumerate(sb_meta):
                    nb = len(blocks)
                    b0 = blocks[0]
                    sa = nb * la
                    sb_ = nb * lb
                    slw = sa + sb_
                    u = gath.tile([P, SLMAX * GW], f32, tag="u")
                    uu = u[:].rearrange("p (s c) -> p s c", c=GW)
                    nc.gpsimd.dma_gather(
                        out_ap=uu[:, 0:sa, :], in_ap=table[0:SPLIT, :],
                        idxs_ap=iA[:, ca0 : ca0 + sa * 8],
                        num_idxs=sa * P, num_idxs_reg=sa * P,
                        elem_size=GW, single_packet=False,
                        queue_num=(2 * si) % 4,
                    )
                    nc.gpsimd.dma_gather(
                        out_ap=uu[:, sa:slw, :], in_ap=table[SPLIT:TROWS, :],
                        idxs_ap=iB[:, cb0 : cb0 + sb_ * 8],
                        num_idxs=sb_ * P, num_idxs_reg=sb_ * P,
                        elem_size=GW, single_packet=False,
                        queue_num=(2 * si + 1) % 4,
                    )
                    ca0 += sa * 8
                    cb0 += sb_ * 8

                    # region views [P, nb, L, C]
                    uA = u[:, 0 : sa * GW].rearrange(
                        "p (b l c) -> p b l c", l=la, c=GW)
                    uB = u[:, sa * GW : slw * GW].rearrange(
                        "p (b l c) -> p b l c", l=lb, c=GW)
                    v = scratch.tile([P, SLMAX * GW], f32, tag="t")
                    vA = v[:, 0 : sa * GW].rearrange(
                        "p (b l c) -> p b l c", l=la, c=GW)
                    vB = v[:, sa * GW : slw * GW].rearrange(
                        "p (b l c) -> p b l c", l=lb, c=GW)
                    lg = small.tile([P, 2 * SLMAX], f32, tag="lg")
                    lgA = lg[:, 0:sa].rearrange("p (b l) -> p b l", l=la)
                    lgB = lg[:, sa:slw].rearrange("p (b l) -> p b l", l=lb)
                    lgA2 = lg[:, SLMAX : SLMAX + sa].rearrange(
                        "p (b l) -> p b l", l=la)
                    lgB2 = lg[:, SLMAX + sa : SLMAX + slw].rearrange(
                        "p (b l) -> p b l", l=lb)
                    wv = small.tile([P, SLMAX], f32, tag="wv")
                    wvA = wv[:, 0:sa].rearrange("p (b l) -> p b l", l=la)
                    wvB = wv[:, sa:slw].rearrange("p (b l) -> p b l", l=lb)
                    dp = small.tile([P, 2 * NBMAX], f32, tag="dp")
                    den = small.tile([P, NBMAX], f32, tag="den")
                    num = small.tile([P, NBMAX * 2 * GW], f32, tag="num")
                    nm = num[:].rearrange("p (b c) -> p b c", c=2 * GW)

                    assert 0 < nposL < C
                    for (uR, vR, lgR, lgR2, wvR, L, r0) in (
                        (uA, vA, lgA, lgA2, wvA, la, 0),
                        (uB, vB, lgB, lgB2, wvB, lb, 1),
                    ):
                        # u += xr (broadcast over L)
                        nc.vector.tensor_add(
                            out=uR[:, :, :, 0:C], in0=uR[:, :, :, 0:C],
                            in1=gem[:, b0 : b0 + nb, None, xr0 : xr0 + C]
                            .to_broadcast([P, nb, L, C]),
                        )
                        # pos channels: v = prelu_.2(5u)  = u>0 ?  5u :  u
                        # neg channels: v = prelu_.2(-5u) = u<0 ? -5u : -u
                        # (alpha stays < 1 so max/branch prelu impls agree;
                        #  parametric_relu lives in the same ACT table as exp)
                        nc.scalar.activation(
                            out=vR[:, :, :, 0:nposL], in_=uR[:, :, :, 0:nposL],
                            func=AF.Prelu, scale=5.0, alpha=0.2,
                        )
                        nc.scalar.activation(
                            out=vR[:, :, :, nposL:C], in_=uR[:, :, :, nposL:C],
                            func=AF.Prelu, scale=-5.0, alpha=0.2,
                        )
                        # logit = sum_c v_pos - sum_c v_neg
                        nc.vector.reduce_sum(
                            out=lgR[:, :, :], in_=vR[:, :, :, 0:nposL],
                            axis=AX.X,
                        )
                        nc.vector.reduce_sum(
                            out=lgR2[:, :, :], in_=vR[:, :, :, nposL:C],
                            axis=AX.X,
                        )
                        nc.vector.tensor_sub(
                            out=lgR[:, :, :], in0=lgR[:, :, :],
                            in1=lgR2[:, :, :],
                        )
                        # w = exp(logit)
                        nc.scalar.activation(
                            out=wvR[:, :, :], in_=lgR[:, :, :], func=AF.Exp,
                        )
                        # den partial = sum_l w
                        nc.vector.reduce_sum(
                            out=dp[:, r0 * NBMAX : r0 * NBMAX + nb],
                            in_=wvR[:, :, :], axis=AX.X,
                        )
                        # vals = u * w  (in place on u)
                        nc.vector.tensor_mul(
                            out=uR[:, :, :, v0 : v0 + C],
                            in0=uR[:, :, :, v0 : v0 + C],
                            in1=wvR[:, :, :, None].to_broadcast([P, nb, L, C]),
                        )
                        # num partial = sum_l vals (strided reduce)
                        nc.vector.reduce_sum(
                            out=nm[:, 0:nb, r0 * GW : r0 * GW + C],
                            in_=uR[:, :, :, v0 : v0 + C]
                            .rearrange("p b l c -> p b c l"),
                            axis=AX.X,
                        )
                    nc.vector.tensor_add(
                        out=den[:, 0:nb], in0=dp[:, 0:nb],
                        in1=dp[:, NBMAX : NBMAX + nb],
                    )
                    nc.vector.tensor_add(
                        out=nm[:, 0:nb, 0:C], in0=nm[:, 0:nb, 0:C],
                        in1=nm[:, 0:nb, GW : GW + C],
                    )
                    out_hook(blocks, den, nm)

            # layer-1 finalize: h = relu(((num - den*xr)*recip)*inv1 + b1)
            def l1_hook(blocks, den, nm):
                b0 = blocks[0]
                nb = len(blocks)
                r = small.tile([P, 2 * NBMAX], f32, tag="rcp")
                nc.vector.tensor_scalar_add(
                    out=r[:, 0:nb], in0=den[:, 0:nb], scalar1=1e-16,
                )
                nc.vector.reciprocal(
                    out=r[:, NBMAX : NBMAX + nb], in_=r[:, 0:nb],
                )
                hsb = small.tile([P, NBMAX * H], f32, tag="hsb")
                hh = hsb[:].rearrange("p (b c) -> p b c", c=H)
                nc.vector.tensor_mul(
                    out=hh[:, 0:nb, :], in0=gem1[:, b0 : b0 + nb, H : 2 * H],
                    in1=den[:, 0:nb, None].to_broadcast([P, nb, H]),
                )
                nc.vector.tensor_sub(
                    out=hh[:, 0:nb, :], in0=nm[:, 0:nb, 0:H],
                    in1=hh[:, 0:nb, :],
                )
                nc.vector.tensor_mul(
                    out=hh[:, 0:nb, :], in0=hh[:, 0:nb, :],
                    in1=r[:, NBMAX : NBMAX + nb, None].to_broadcast([P, nb, H]),
                )
                nc.vector.tensor_mul(
                    out=hh[:, 0:nb, :], in0=hh[:, 0:nb, :],
                    in1=inv1_t[:, None, :].to_broadcast([P, nb, H]),
                )
                nc.vector.tensor_add(
                    out=hh[:, 0:nb, :], in0=hh[:, 0:nb, :],
                    in1=b1_t[:, None, :].to_broadcast([P, nb, H]),
                )
                nc.scalar.activation(
                    out=hsb[:, 0 : nb * H], in_=hsb[:, 0 : nb * H],
                    func=AF.Relu,
                )
                for bi, b in enumerate(blocks):
                    pst = psum.tile([H, P], f32, tag="tr_ps")
                    nc.tensor.transpose(
                        out=pst[:], in_=hh[:, bi, :], identity=ident[:],
                    )
                    nc.vector.tensor_copy(
                        out=hT[:, b * P : (b + 1) * P], in_=pst[:],
                    )
                    ps2 = psum.tile([P, 3 * O], f32, tag="gemm_ps")
                    nc.tensor.matmul(
                        out=ps2[:], lhsT=hT[:, b * P : (b + 1) * P],
                        rhs=w2[:], start=True, stop=True,
                    )
                    nc.vector.tensor_copy(out=gem2[:, b, :], in_=ps2[:])

            edge_phase(1, table1, gem1, H, npos1, l1_hook)

            # ---------------- AllGather table2 (gem2 built inside l1_hook)
            nc.sync.dma_start(
                out=t2_in.ap()[0 : PAD_BLK * P, :]
                .rearrange("(b p) h -> p b h", p=P),
                in_=gem2[:, 0:PAD_BLK, 0 : 2 * O],
            )
            nc.sync.dma_start(
                out=t2_in.ap()[PAD_BLK * P : NPC, :],
                in_=gem2[0:PAD_P0, PAD_BLK, 0 : 2 * O],
            )
            nc.sync.dma_start(
                out=t2_in.ap()[NPC:NPC_PAD, :],
                in_=poison[0:NPAD, GW : GW + 2 * O],
            )
            nc.gpsimd.collective_compute(
                "AllGather", mybir.AluOpType.bypass, replica_groups=groups,
                ins=[t2_in[:].opt()], outs=[table2[:].opt()],
            )

            # layer-2 finalize: alpha = softplus(num*recip + b2) + 1e-6
            osb = keep.tile([P, NBLK, O], f32)

            def l2_hook(blocks, den, nm):
                b0 = blocks[0]
                nb = len(blocks)
                r = small.tile([P, 2 * NBMAX], f32, tag="rcp")
                nc.vector.tensor_scalar_add(
                    out=r[:, 0:nb], in0=den[:, 0:nb], scalar1=1e-16,
                )
                nc.vector.reciprocal(
                    out=r[:, NBMAX : NBMAX + nb], in_=r[:, 0:nb],
                )
                nc.vector.tensor_mul(
                    out=osb[:, b0 : b0 + nb, :], in0=nm[:, 0:nb, 0:O],
                    in1=r[:, NBMAX : NBMAX + nb, None].to_broadcast([P, nb, O]),
                )

            edge_phase(2, table2, gem2, O, npos2, l2_hook)

            # alpha = softplus(osb + b2) + 1e-6, one fused tail
            nc.vector.tensor_add(
                out=osb[:], in0=osb[:],
                in1=b2_t[:, None, :].to_broadcast([P, NBLK, O]),
            )
            oflat = osb[:].rearrange("p b c -> p (b c)")
            nc.scalar.activation(out=oflat, in_=oflat, func=AF.Exp)
            nc.scalar.activation(out=oflat, in_=oflat, func=AF.Ln, bias=1.0)
            nc.vector.tensor_scalar_add(out=oflat, in0=oflat, scalar1=1e-6)
            nc.sync.dma_start(
                out=alpha_d.ap().rearrange("(b p) o -> p b o", p=P),
                in_=osb[:],
            )

    nc.compile()
    return nc


# ---------------------------------------------------------------- runner
def _run(inputs, N, IN_C, H, O, slmax=104, sim=False, trace=False):
    x = np.asarray(inputs["x"], np.float32)
    ei = np.asarray(inputs["edge_index"])
    plan = _plan(ei, N, slmax)
    wp = _prep_weights(
        np.asarray(inputs["W1_l"], np.float32),
        np.asarray(inputs["W1_r"], np.float32),
        np.asarray(inputs["att1"], np.float32),
        np.asarray(inputs["b1"], np.float32),
        np.asarray(inputs["W2_l"], np.float32),
        np.asarray(inputs["W2_r"], np.float32),
        np.asarray(inputs["att2"], np.float32),
        np.asarray(inputs["b2"], np.float32),
    )
    nc = _build(plan, wp, IN_C, H, O, debug=sim)

    pos, NPC_PAD = plan["pos"], plan["NPC_PAD"]
    xT_full = np.zeros((IN_C, CORES * NPC_PAD), np.float32)
    xT_full[:, pos] = x.T
    in_maps = []
    for k in range(CORES):
        in_maps.append({
            "xT": np.ascontiguousarray(
                xT_full[:, k * NPC_PAD : (k + 1) * NPC_PAD]),
            "W1cat": wp["W1cat"], "W2cat": wp["W2cat"],
            "inv1": np.tile(wp["inv1"][None, :], (P, 1)),
            "b1": np.tile(wp["b1"][None, :], (P, 1)),
            "b2": np.tile(wp["b2"][None, :], (P, 1)),
            "idxA": plan["idxA"][k], "idxB": plan["idxB"][k],
        })

    if sim:
        from concourse import bass_interp
        msim = bass_interp.MultiCoreSim(nc, CORES)
        for k in range(CORES):
            for name, arr in in_maps[k].items():
                msim.cores[k].tensor(name)[:] = arr
        msim.simulate()
        slabs = [msim.cores[k].mem_tensor("alpha").reshape(NPC_PAD, O)
                 for k in range(CORES)]
        exec_ns = None
    else:
        from concourse.bass_utils import run_bass_kernel_spmd
        res = run_bass_kernel_spmd(nc, in_maps, list(range(CORES)),
                                   trace=trace)
        slabs = [np.asarray(res.results[k]["alpha"]).reshape(NPC_PAD, O)
                 for k in range(CORES)]
        exec_ns = res.exec_time_ns
        if trace and exec_ns is None:
            # NTFF hook unavailable: wall-clock repeated executions
            import time
            times = []
            for _ in range(4):
                t0 = time.perf_counter()
                run_bass_kernel_spmd(nc, in_maps, list(range(CORES)))
                times.append(time.perf_counter() - t0)
            exec_ns = int(min(times) * 1e9)
            print("wall-clock times (s):", [f"{t:.3f}" for t in times])

    full = np.concatenate(slabs, axis=0)
    out = full[pos].astype(np.float32)
    return out, exec_ns


def kernel(**inputs) -> np.ndarray:
    out, _ = _run(inputs, N=50000, IN_C=128, H=64, O=32)
    return out
